# revision 9
# baseline (speedup 1.0000x reference)
"""CapsNet forward pass on 8 Trainium2 NeuronCores (Bass/Tile).

Data-parallel over batch (256 -> 32/core). Weights replicated. Per routing
iteration the b_ij update needs a full-batch mean -> AllReduce of [1152,10];
the decoder's legacy batch-softmax needs an AllReduce of [1,10].

Key formulations (avoid materializing u_hat [B,1152,10,16]):
  s_j[b,(c,o)]   = sum_flat u[b,flat] * (c[flat//8, c] * W_s[flat,(c,o)])
  delta_b[r,c]   = sum_{i,o} W_s[(r,i),(c,o)] * M[(r,i),(c,o)],
  M[flat,(c,o)]  = (1/B) sum_b u[b,flat] * v[b,(c,o)]   (one matmul per k-tile)
where flat = co*36 + pos = r*8 + i  (the reference's reshape ordering).
"""
import sys
sys.path.insert(0, '/opt/trn_rl_repo')

import numpy as np

B = 256
NCORES = 8
BC = B // NCORES          # 32 per core
ROUTES = 1152
NCAPS = 10
DIN = 8
DOUT = 16
NT = 72                   # k-tiles of 128 over flat=9216
CO = NCAPS * DOUT         # 160

_CACHE = {}


def _build_nc():
    import concourse.bacc as bacc
    import concourse.mybir as mybir
    from concourse import tile

    F32 = mybir.dt.float32
    F32R = mybir.dt.float32r
    AX = mybir.AxisListType
    ALU = mybir.AluOpType
    ACT = mybir.ActivationFunctionType

    nc = bacc.Bacc("TRN2", target_bir_lowering=False, debug=False,
                   num_devices=NCORES)

    # ---------------- external inputs (per-core) ----------------
    x_in = nc.dram_tensor("x", [BC, 784], F32R, kind="ExternalInput")
    c1w_in = nc.dram_tensor("c1w", [81, 256], F32R, kind="ExternalInput")
    c1b_in = nc.dram_tensor("c1b", [128, 2], F32, kind="ExternalInput")
    w2_in = nc.dram_tensor("w2", [81, 2, 128, 256], F32R, kind="ExternalInput")
    p2b_in = nc.dram_tensor("p2b", [128, 2], F32, kind="ExternalInput")
    ws_in = nc.dram_tensor("ws", [NT, 128, CO], F32R, kind="ExternalInput")
    e8r_in = nc.dram_tensor("e8r", [128, 128], F32, kind="ExternalInput")
    e8_in = nc.dram_tensor("e8", [128, 16], F32, kind="ExternalInput")
    sel_in = nc.dram_tensor("sel16", [16, 128], F32, kind="ExternalInput")
    o1632_in = nc.dram_tensor("o1632", [16, 32], F32, kind="ExternalInput")
    o321_in = nc.dram_tensor("o321", [32, 2], F32, kind="ExternalInput")
    o132_in = nc.dram_tensor("o132", [1, 32], F32, kind="ExternalInput")
    iden_in = nc.dram_tensor("iden", [128, 128], F32, kind="ExternalInput")
    d1w_in = nc.dram_tensor("d1w", [160, 512], F32R, kind="ExternalInput")
    d1b_in = nc.dram_tensor("d1b", [128, 4], F32, kind="ExternalInput")
    d2w_in = nc.dram_tensor("d2w", [512, 1024], F32R, kind="ExternalInput")
    d2b_in = nc.dram_tensor("d2b", [128, 8], F32, kind="ExternalInput")
    d3w_in = nc.dram_tensor("d3w", [1024, 896], F32R, kind="ExternalInput")
    d3b_in = nc.dram_tensor("d3b", [128, 7], F32, kind="ExternalInput")

    out_v = nc.dram_tensor("out_v", [BC, CO], F32, kind="ExternalOutput")
    out_recon = nc.dram_tensor("out_recon", [BC, 784], F32, kind="ExternalOutput")
    out_masked = nc.dram_tensor("out_masked", [BC, NCAPS], F32, kind="ExternalOutput")

    CORE_IDS = list(range(NCORES))
    BCH = [(0, 12), (12, 12), (24, 8)]          # conv2 batch chunks
    SQCH = [(0, 512), (512, 512), (1024, 512), (1536, 512), (2048, 256)]

    with tile.TileContext(nc) as tc:
        with tc.tile_pool(name="const", bufs=1) as cp, \
             tc.tile_pool(name="dram", bufs=1, space="DRAM") as dp, \
             tc.tile_pool(name="outer", bufs=1) as op:

            # constants (~2KB/part)
            c1w = cp.tile([81, 256], F32R); nc.sync.dma_start(c1w[:], c1w_in[:])
            c1b = cp.tile([128, 2], F32); nc.sync.dma_start(c1b[:], c1b_in[:])
            p2b = cp.tile([128, 2], F32); nc.sync.dma_start(p2b[:], p2b_in[:])
            e8r = cp.tile([128, 128], F32); nc.sync.dma_start(e8r[:], e8r_in[:])
            e8 = cp.tile([128, 16], F32); nc.sync.dma_start(e8[:], e8_in[:])
            sel16 = cp.tile([16, 128], F32); nc.sync.dma_start(sel16[:], sel_in[:])
            o1632 = cp.tile([16, 32], F32); nc.sync.dma_start(o1632[:], o1632_in[:])
            o321 = cp.tile([32, 2], F32); nc.sync.dma_start(o321[:], o321_in[:])
            o132 = cp.tile([1, 32], F32); nc.sync.dma_start(o132[:], o132_in[:])
            iden = cp.tile([128, 128], F32); nc.sync.dma_start(iden[:], iden_in[:])
            idenr = cp.tile([128, 128], F32R)
            nc.vector.tensor_copy(idenr[:], iden[:])

            # DRAM intermediates
            u_lin = dp.tile([9216, BC], F32R)     # pre-squash, flat-major
            ar_in0 = dp.tile([ROUTES, NCAPS], F32)
            ar_out0 = dp.tile([ROUTES, NCAPS], F32)
            ar_in1 = dp.tile([ROUTES, NCAPS], F32)
            ar_out1 = dp.tile([ROUTES, NCAPS], F32)
            z2_in = dp.tile([1, NCAPS], F32)
            z2_out = dp.tile([1, NCAPS], F32)

            # ============ conv phase ============
            with tc.tile_pool(name="hpool", bufs=1) as hp:
                h = [hp.tile([128, BC * 400], F32R, tag=f"h{i}", name=f"h{i}")
                     for i in range(2)]
                u_sb = [hp.tile([128, 1152], F32R, tag=f"usb{i}", name=f"usb{i}")
                        for i in range(2)]

                # conv1: im2col P1 [81, BC*400], then 2 m-tiles x 32 b-chunks
                with tc.tile_pool(name="p1pool", bufs=1) as p1p, \
                     tc.tile_pool(name="psC1", bufs=4, space="PSUM") as ps1p:
                    p1 = p1p.tile([81, BC * 400], F32R)
                    xv = x_in[:].rearrange("b (y x) -> b y x", y=28, x=28)
                    for ky in range(9):
                        for kx in range(9):
                            k = ky * 9 + kx
                            nc.sync.dma_start(
                                p1[k:k + 1, :],
                                xv[:, ky:ky + 20, kx:kx + 20].unsqueeze(0))
                    for mt in range(2):
                        for b0 in range(BC):
                            psc = ps1p.tile([128, 400], F32, tag="c1ps",
                                            name="c1ps")
                            nc.tensor.matmul(psc[:],
                                             c1w[:, mt * 128:(mt + 1) * 128],
                                             p1[:, b0 * 400:(b0 + 1) * 400],
                                             start=True, stop=True)
                            nc.scalar.activation(
                                h[mt][:, b0 * 400:(b0 + 1) * 400],
                                psc[:], ACT.Relu,
                                bias=c1b[:, mt:mt + 1], scale=1.0)

                # W_s prefetch (P1 freed; lives until end of routing)
                ws_sb = op.tile([128, NT * CO], F32R)
                nc.sync.dma_start(
                    ws_sb[:].rearrange("p (t c) -> p t c", t=NT, c=CO),
                    ws_in[:].transpose([1, 0, 2]))

                # conv2: 81 taps x 2 ci-tiles accumulate into 6 psum tiles
                with tc.tile_pool(name="w2pool", bufs=4) as w2p, \
                     tc.tile_pool(name="psC2", bufs=1, space="PSUM") as ps2p:
                    psc2 = {}
                    for co_t in range(2):
                        for ci, (b0, nb) in enumerate(BCH):
                            psc2[(co_t, ci)] = ps2p.tile(
                                [128, nb * 36], F32,
                                tag=f"c2ps{co_t}{ci}", name=f"c2ps{co_t}{ci}")
                    hv = [h[i][:].rearrange(
                        "p (b y2 s x2 t) -> p b y2 s x2 t",
                        b=BC, y2=10, s=2, x2=10, t=2) for i in range(2)]
                    for tap in range(81):
                        ky, kx = divmod(tap, 9)
                        for ci_t in range(2):
                            w2t = w2p.tile([128, 256], F32R, tag="w2t",
                                           name="w2t")
                            nc.sync.dma_start(w2t[:], w2_in[tap, ci_t])
                            for co_t in range(2):
                                for ci, (b0, nb) in enumerate(BCH):
                                    rhs = hv[ci_t][:, b0:b0 + nb,
                                                   ky // 2: ky // 2 + 6, ky % 2,
                                                   kx // 2: kx // 2 + 6, kx % 2]
                                    rhs = rhs.transpose([0, 2, 3, 1])
                                    nc.tensor.matmul(
                                        psc2[(co_t, ci)],
                                        w2t[:, co_t * 128:(co_t + 1) * 128],
                                        rhs,
                                        start=(tap == 0 and ci_t == 0),
                                        stop=(tap == 80 and ci_t == 1))
                    for co_t in range(2):
                        for ci, (b0, nb) in enumerate(BCH):
                            dst = u_sb[co_t][:].rearrange(
                                "p (pos b) -> p pos b", pos=36, b=BC)[:, :, b0:b0 + nb]
                            srcv = psc2[(co_t, ci)][:].rearrange(
                                "p (pos b) -> p pos b", pos=36, b=nb)
                            nc.scalar.activation(
                                dst, srcv, ACT.Identity,
                                bias=p2b[:, co_t:co_t + 1], scale=1.0)
                    ulv = u_lin[:].rearrange("(ct p pos) b -> ct p pos b",
                                             ct=2, p=128, pos=36)
                    for co_t in range(2):
                        nc.sync.dma_start(
                            ulv[co_t],
                            u_sb[co_t][:].rearrange("p (pos b) -> p pos b",
                                                    pos=36, b=BC))
            # h, u_sb freed

            # ============ routing phase ============
            with tc.tile_pool(name="rps", bufs=1) as rps, \
                 tc.tile_pool(name="cwp", bufs=3) as cwp, \
                 tc.tile_pool(name="prp", bufs=3) as prp:

                # small long-lived tiles
                v_sb = rps.tile([BC, CO], F32)
                v_r = rps.tile([BC, CO], F32R)
                s_sb = rps.tile([BC, CO], F32)
                invz = rps.tile([BC, NCAPS], F32)

                with tc.tile_pool(name="rheavy", bufs=1) as rh:
                    u_tq = rh.tile([128, NT * BC], F32R)
                    # --- load u_T + squash (scratch pool closes after) ---
                    with tc.tile_pool(name="sqp", bufs=1) as sqp, \
                         tc.tile_pool(name="psSq", bufs=2, space="PSUM") as psq:
                        u_t = sqp.tile([128, NT * BC], F32R)
                        nc.sync.dma_start(
                            u_t[:].rearrange("p (t b) -> p t b", t=NT, b=BC),
                            u_lin[:].rearrange("(t p) b -> p t b", t=NT, p=128))
                        sq = sqp.tile([128, NT * BC], F32)
                        nc.vector.tensor_mul(sq[:], u_t[:], u_t[:])
                        fac = sqp.tile([128, NT * BC], F32)
                        for off, ln in SQCH:
                            pss = psq.tile([128, ln], F32, tag="sqps",
                                           name="sqps")
                            nc.tensor.matmul(pss[:], e8r[:], sq[:, off:off + ln],
                                             start=True, stop=True)
                            t1 = sqp.tile([128, ln], F32, tag="sq1", name="sq1",
                                          bufs=2)
                            nc.scalar.activation(t1[:], pss[:], ACT.Sqrt)
                            t2 = sqp.tile([128, ln], F32, tag="sq2", name="sq2",
                                          bufs=2)
                            nc.vector.tensor_scalar_add(t2[:], pss[:], 1.0)
                            nc.vector.reciprocal(t2[:], t2[:])
                            nc.vector.tensor_mul(fac[:, off:off + ln],
                                                 t1[:], t2[:])
                        nc.vector.tensor_mul(u_tq[:], u_t[:], fac[:])
                        # u_b[b, flat] via 72 on-chip transposes of u_tq k-tiles
                        u_b = rh.tile([BC, 9216], F32R)
                        for t in range(NT):
                            pstu = psq.tile([BC, 128], F32R, tag="trub",
                                            name="trub", bufs=2)
                            nc.tensor.transpose(
                                pstu[:], u_tq[:, t * BC:(t + 1) * BC], idenr[:])
                            nc.vector.tensor_copy(
                                u_b[:, t * 128:(t + 1) * 128], pstu[:])

                    b_sb = rh.tile([16, NT * NCAPS], F32)
                    nc.vector.memset(b_sb[:], 0.0)
                    eb = rh.tile([16, NT * NCAPS], F32)
                    z16 = rh.tile([16, NCAPS], F32)
                    wsv = ws_sb[:].rearrange("p (t c o) -> t p c o",
                                             t=NT, c=NCAPS, o=DOUT)

                    psR_cm = tc.tile_pool(name="psRoute", bufs=1, space="PSUM")
                    psR = psR_cm.__enter__()

                    def softmax_c():
                        nc.scalar.activation(eb[:], b_sb[:], ACT.Exp)
                        nc.vector.tensor_reduce(
                            z16[:],
                            eb[:].rearrange("p (t c) -> p c t", t=NT, c=NCAPS),
                            AX.X, ALU.add)
                        psz = psR.tile([32, NCAPS], F32, tag="zps", name="zps")
                        nc.tensor.matmul(psz[:], o1632[:], z16[:],
                                         start=True, stop=True)
                        nc.vector.reciprocal(invz[:], psz[:])
                        pse = [psR.tile([128, 360], F32, tag=f"ebps{i}",
                                        name=f"ebps{i}") for i in range(2)]
                        for i in range(2):
                            nc.tensor.matmul(pse[i][:], sel16[:],
                                             eb[:, i * 360:(i + 1) * 360],
                                             start=True, stop=True)
                        return pse

                    def sj_pass(itr, pse):
                        pss = psR.tile([BC, CO], F32, tag="sps", name="sps")
                        for t in range(NT):
                            if itr == 0:
                                rhs = ws_sb[:, t * CO:(t + 1) * CO]
                            else:
                                cw = cwp.tile([128, CO], F32R, tag="cw",
                                              name="cw")
                                ebr = pse[t // 36][:, (t % 36) * 10:
                                                   (t % 36) * 10 + 10]
                                nc.vector.tensor_tensor(
                                    cw[:].rearrange("p (c o) -> p c o",
                                                    c=NCAPS, o=DOUT),
                                    wsv[t],
                                    ebr.unsqueeze(2).broadcast_to(
                                        [128, NCAPS, DOUT]),
                                    ALU.mult)
                                rhs = cw[:]
                            nc.tensor.matmul(pss[:],
                                             u_tq[:, t * BC:(t + 1) * BC], rhs,
                                             start=(t == 0), stop=(t == NT - 1))
                        if itr == 0:
                            nc.vector.tensor_scalar_mul(s_sb[:], pss[:],
                                                        1.0 / ROUTES)
                        else:
                            nc.vector.tensor_tensor(
                                s_sb[:].rearrange("p (c o) -> p c o",
                                                  c=NCAPS, o=DOUT),
                                pss[:].rearrange("p (c o) -> p c o",
                                                 c=NCAPS, o=DOUT),
                                invz[:].unsqueeze(2).broadcast_to(
                                    [BC, NCAPS, DOUT]),
                                ALU.mult)
                        # v = s*|s|/(1+s^2)
                        t1 = prp.tile([BC, CO], F32, tag="v1", name="v1")
                        nc.vector.tensor_mul(t1[:], s_sb[:], s_sb[:])
                        nc.vector.tensor_scalar_add(t1[:], t1[:], 1.0)
                        nc.vector.reciprocal(t1[:], t1[:])
                        t2 = prp.tile([BC, CO], F32, tag="v2", name="v2")
                        nc.scalar.activation(t2[:], s_sb[:], ACT.Abs)
                        nc.vector.tensor_mul(t2[:], t2[:], s_sb[:])
                        nc.vector.tensor_mul(v_sb[:], t2[:], t1[:])
                        nc.scalar.activation(v_r[:], v_sb[:], ACT.Copy)

                    def b_update(arin, arout):
                        psd = [psR.tile([16, 360], F32, tag=f"dps{i}",
                                        name=f"dps{i}") for i in range(2)]
                        for t in range(NT):
                            psm = psR.tile([128, CO], F32, tag="mps",
                                           name="mps", bufs=2)
                            nc.tensor.matmul(psm[:],
                                             u_b[:, t * 128:(t + 1) * 128],
                                             v_r[:], start=True, stop=True)
                            pr = prp.tile([128, NCAPS], F32, tag="pr",
                                          name="pr")
                            sc = cwp.tile([128, CO], F32, tag="sc", name="sc")
                            nc.vector.scalar_tensor_tensor(
                                sc[:].rearrange("p (c o) -> p c o",
                                                c=NCAPS, o=DOUT),
                                psm[:].rearrange("p (c o) -> p c o",
                                                 c=NCAPS, o=DOUT),
                                1.0 / B, wsv[t], ALU.mult, ALU.mult)
                            nc.vector.tensor_reduce(
                                pr[:],
                                sc[:].rearrange("p (c o) -> p c o",
                                                c=NCAPS, o=DOUT),
                                AX.X, ALU.add)
                            nc.tensor.matmul(
                                psd[t // 36][:, (t % 36) * 10:(t % 36) * 10 + 10],
                                e8[:], pr[:], start=True, stop=True)
                        dsb = rh.tile([16, NT * NCAPS], F32, tag="dsb",
                                      name="dsb")
                        for i in range(2):
                            nc.vector.tensor_copy(dsb[:, i * 360:(i + 1) * 360],
                                                  psd[i][:])
                        nc.sync.dma_start(
                            arin[:].rearrange("(t rl) c -> rl t c",
                                              t=NT, rl=16),
                            dsb[:].rearrange("p (t c) -> p t c",
                                             t=NT, c=NCAPS))
                        nc.gpsimd.collective_compute(
                            "AllReduce", ALU.add, replica_groups=[CORE_IDS],
                            ins=[arin.opt()], outs=[arout.opt()])
                        dsb2 = rh.tile([16, NT * NCAPS], F32, tag="dsb2",
                                       name="dsb2")
                        nc.sync.dma_start(
                            dsb2[:].rearrange("p (t c) -> p t c",
                                              t=NT, c=NCAPS),
                            arout[:].rearrange("(t rl) c -> rl t c",
                                               t=NT, rl=16))
                        nc.vector.tensor_add(b_sb[:], b_sb[:], dsb2[:])

                    sj_pass(0, None)
                    b_update(ar_in0, ar_out0)
                    pse = softmax_c()
                    sj_pass(1, pse)
                    b_update(ar_in1, ar_out1)
                    pse = softmax_c()
                    sj_pass(2, pse)

                    nc.sync.dma_start(out_v[:], v_sb[:])
                    psR_cm.__exit__(None, None, None)
                # rheavy freed (u_tq, u_b, b/eb)

                # ============ classes / mask / decoder ============
                with tc.tile_pool(name="dec", bufs=1) as dcp, \
                     tc.tile_pool(name="psDec", bufs=1, space="PSUM") as psD:
                    cls = dcp.tile([BC, NCAPS], F32)
                    t1 = prp.tile([BC, CO], F32, tag="v1", name="v1")
                    nc.vector.tensor_mul(t1[:], v_sb[:], v_sb[:])
                    nc.vector.tensor_reduce(
                        cls[:],
                        t1[:].rearrange("p (c o) -> p c o", c=NCAPS, o=DOUT),
                        AX.X, ALU.add)
                    nc.scalar.activation(cls[:], cls[:], ACT.Sqrt)
                    ecls = dcp.tile([BC, NCAPS], F32)
                    nc.scalar.activation(ecls[:], cls[:], ACT.Exp)
                    psz2 = psD.tile([1, NCAPS], F32, tag="z2ps", name="z2ps")
                    nc.tensor.matmul(psz2[:], o321[:, 0:1], ecls[:],
                                     start=True, stop=True)
                    z2sb = dcp.tile([1, NCAPS], F32)
                    nc.vector.tensor_copy(z2sb[:], psz2[:])
                    nc.sync.dma_start(z2_in[:], z2sb[:])
                    nc.gpsimd.collective_compute(
                        "AllReduce", ALU.add, replica_groups=[CORE_IDS],
                        ins=[z2_in.opt()], outs=[z2_out.opt()])
                    z2l = dcp.tile([1, NCAPS], F32)
                    nc.sync.dma_start(z2l[:], z2_out[:])
                    psz3 = psD.tile([BC, NCAPS], F32, tag="z3ps", name="z3ps")
                    nc.tensor.matmul(psz3[:], o132[:], z2l[:],
                                     start=True, stop=True)
                    q = dcp.tile([BC, NCAPS], F32)
                    nc.vector.reciprocal(q[:], psz3[:])
                    nc.vector.tensor_mul(q[:], q[:], ecls[:])
                    qm = dcp.tile([BC, 2], F32)
                    nc.vector.tensor_reduce(qm[:, 0:1], q[:], AX.X, ALU.max)
                    masked = dcp.tile([BC, NCAPS], F32)
                    nc.vector.tensor_tensor(masked[:], q[:],
                                            qm[:, 0:1].broadcast_to([BC, NCAPS]),
                                            ALU.is_ge)
                    nc.sync.dma_start(out_masked[:], masked[:])

                    # flat = v * masked -> [160, 32]
                    flat = dcp.tile([BC, CO], F32)
                    nc.vector.tensor_tensor(
                        flat[:].rearrange("p (c o) -> p c o", c=NCAPS, o=DOUT),
                        v_sb[:].rearrange("p (c o) -> p c o", c=NCAPS, o=DOUT),
                        masked[:].unsqueeze(2).broadcast_to([BC, NCAPS, DOUT]),
                        ALU.mult)
                    pst1 = psD.tile([128, BC], F32, tag="tr1", name="tr1")
                    pst2 = psD.tile([32, BC], F32, tag="tr2", name="tr2")
                    nc.tensor.transpose(pst1[:], flat[:, 0:128],
                                        iden[0:BC, 0:BC])
                    nc.tensor.transpose(pst2[:], flat[:, 128:160],
                                        iden[0:BC, 0:BC])
                    flt_a = dcp.tile([128, BC], F32R)
                    flt_b = dcp.tile([32, BC], F32R)
                    nc.vector.tensor_copy(flt_a[:], pst1[:])
                    nc.vector.tensor_copy(flt_b[:], pst2[:])

                    d1w = dcp.tile([128, 2 * 512], F32R)
                    nc.sync.dma_start(d1w[:, 0:512], d1w_in[0:128, :])
                    nc.sync.dma_start(d1w[0:32, 512:1024], d1w_in[128:160, :])
                    d1b = dcp.tile([128, 4], F32)
                    nc.sync.dma_start(d1b[:], d1b_in[:])
                    d2b = dcp.tile([128, 8], F32)
                    nc.sync.dma_start(d2b[:], d2b_in[:])
                    d3b = dcp.tile([128, 7], F32)
                    nc.sync.dma_start(d3b[:], d3b_in[:])
                    d2w = [dcp.tile([128, 1024], F32R, tag=f"d2w{k}",
                                    name=f"d2w{k}") for k in range(4)]
                    for k in range(4):
                        nc.sync.dma_start(d2w[k][:],
                                          d2w_in[k * 128:(k + 1) * 128, :])
                    d3w = [dcp.tile([128, 896], F32R, tag=f"d3w{k}",
                                    name=f"d3w{k}") for k in range(8)]
                    for k in range(8):
                        nc.sync.dma_start(d3w[k][:],
                                          d3w_in[k * 128:(k + 1) * 128, :])

                    r1 = [dcp.tile([128, BC], F32R, tag=f"r1{m}",
                                   name=f"r1{m}") for m in range(4)]
                    for m in range(4):
                        psd1 = psD.tile([128, BC], F32, tag="d1ps", name="d1ps")
                        nc.tensor.matmul(psd1[:],
                                         d1w[:, m * 128:(m + 1) * 128],
                                         flt_a[:], start=True, stop=False)
                        nc.tensor.matmul(
                            psd1[:],
                            d1w[0:32, 512 + m * 128:512 + (m + 1) * 128],
                            flt_b[:], start=False, stop=True)
                        nc.scalar.activation(r1[m][:], psd1[:], ACT.Relu,
                                             bias=d1b[:, m:m + 1], scale=1.0)
                    r2 = [dcp.tile([128, BC], F32R, tag=f"r2{m}",
                                   name=f"r2{m}") for m in range(8)]
                    for m in range(8):
                        psd2 = psD.tile([128, BC], F32, tag="d2ps", name="d2ps")
                        for k in range(4):
                            nc.tensor.matmul(psd2[:],
                                             d2w[k][:, m * 128:(m + 1) * 128],
                                             r1[k][:], start=(k == 0),
                                             stop=(k == 3))
                        nc.scalar.activation(r2[m][:], psd2[:], ACT.Relu,
                                             bias=d2b[:, m:m + 1], scale=1.0)
                    recon = dcp.tile([BC, 784], F32)
                    for m in range(7):
                        mp = 128 if m < 6 else 16
                        psd3 = psD.tile([128, BC], F32, tag="d3ps", name="d3ps")
                        for k in range(8):
                            nc.tensor.matmul(psd3[0:mp, :],
                                             d3w[k][:, m * 128:m * 128 + mp],
                                             r2[k][:], start=(k == 0),
                                             stop=(k == 7))
                        rt = prp.tile([128, BC], F32, tag="rt", name="rt")
                        nc.scalar.activation(rt[0:mp, :], psd3[0:mp, :],
                                             ACT.Sigmoid,
                                             bias=d3b[0:mp, m:m + 1], scale=1.0)
                        pstr = psD.tile([BC, 128], F32, tag="trr", name="trr")
                        nc.tensor.transpose(pstr[:, 0:mp], rt[0:mp, :],
                                            iden[0:mp, 0:mp])
                        nc.vector.tensor_copy(recon[:, m * 128:m * 128 + mp],
                                              pstr[:, 0:mp])
                    nc.sync.dma_start(out_recon[:], recon[:])

    nc.compile()
    return nc


def _host_consts():
    p = np.arange(128)
    e8r = (p[:, None] // 8 == p[None, :] // 8).astype(np.float32)
    e8 = (p[:, None] // 8 == np.arange(16)[None, :]).astype(np.float32)
    sel16 = (np.arange(16)[:, None] == p[None, :] // 8).astype(np.float32)
    o1632 = np.ones((16, 32), np.float32)
    o321 = np.ones((32, 2), np.float32)
    o132 = np.ones((1, 32), np.float32)
    iden = np.eye(128, dtype=np.float32)
    return e8r, e8, sel16, o1632, o321, o132, iden


def kernel(x, conv1_w, conv1_b, prim_w, prim_b, W_dig,
           d1_w, d1_b, d2_w, d2_b, d3_w, d3_b):
    from concourse.bass_utils import run_bass_kernel_spmd

    if "nc" not in _CACHE:
        _CACHE["nc"] = _build_nc()
    nc = _CACHE["nc"]

    f32 = np.float32
    x = np.ascontiguousarray(np.asarray(x, f32).reshape(B, 784))
    c1w = np.ascontiguousarray(np.asarray(conv1_w, f32).reshape(256, 81).T)
    c1b = np.ascontiguousarray(np.asarray(conv1_b, f32).reshape(2, 128).T)
    # prim_w [256co, 256ci, 9, 9] -> [81(tap), 2(ci_t), 128(ci_p), 256(co)]
    w2 = np.asarray(prim_w, f32).transpose(2, 3, 1, 0).reshape(81, 2, 128, 256)
    w2 = np.ascontiguousarray(w2)
    p2b = np.ascontiguousarray(np.asarray(prim_b, f32).reshape(2, 128).T)
    ws = np.asarray(W_dig, f32).transpose(0, 3, 1, 2).reshape(9216, CO)
    ws = np.ascontiguousarray(ws.reshape(NT, 128, CO))
    d1w = np.ascontiguousarray(np.asarray(d1_w, f32).T)           # [160, 512]
    d1b = np.ascontiguousarray(np.asarray(d1_b, f32).reshape(4, 128).T)
    d2w = np.ascontiguousarray(np.asarray(d2_w, f32).T)           # [512, 1024]
    d2b = np.ascontiguousarray(np.asarray(d2_b, f32).reshape(8, 128).T)
    d3w = np.zeros((1024, 896), f32)
    d3w[:, 0:784] = np.asarray(d3_w, f32).T
    d3b = np.zeros((896,), f32)
    d3b[0:784] = np.asarray(d3_b, f32)
    d3b = np.ascontiguousarray(d3b.reshape(7, 128).T)
    e8r, e8, sel16, o1632, o321, o132, iden = _host_consts()

    shared = dict(c1w=c1w, c1b=c1b, w2=w2, p2b=p2b, ws=ws,
                  e8r=e8r, e8=e8, sel16=sel16, o1632=o1632, o321=o321,
                  o132=o132, iden=iden, d1w=d1w, d1b=d1b, d2w=d2w, d2b=d2b,
                  d3w=d3w, d3b=d3b)
    in_maps = [dict(shared, x=np.ascontiguousarray(x[i * BC:(i + 1) * BC]))
               for i in range(NCORES)]

    res = run_bass_kernel_spmd(nc, in_maps, list(range(NCORES))).results

    output = np.empty((B, NCAPS, DOUT, 1), f32)
    recon = np.empty((B, 1, 28, 28), f32)
    masked = np.empty((B, NCAPS), f32)
    for i in range(NCORES):
        sl = slice(i * BC, (i + 1) * BC)
        output[sl] = res[i]["out_v"].reshape(BC, NCAPS, DOUT, 1)
        recon[sl] = res[i]["out_recon"].reshape(BC, 1, 28, 28)
        masked[sl] = res[i]["out_masked"]
    return output, recon, masked
